# revision 3
# baseline (speedup 1.0000x reference)
"""HardHeatMap Trainium2 kernel, v2 (3x3-blob matmul scatter).

Identity: scatter 1.0 at floor(boxes*4096) then 3x3/stride-1/pad-1 max
pool == every point paints a 3x3 blob (clipped at edges); a pixel is 1
iff some blob covers it.

Device algorithm per core (band of 512 rows = 4 chunks of 128 rows x
8 col-blocks of 512): each point is ONE matmul K-slot.  For a K-tile of
up to 128 points:
    ap3[k, p] = (|p - px_k| <= 1)   row window, HOST-prepared fp16 lhsT
    ay3[k, y] = (|y - cy_k| <= 1)   col window, built on device (rhs)
    psum[p, y] += ap3^T @ ay3       3x3 blob counts (f32)
ay3 is built two ways, interleaved to balance engines: 2/3 of K-tiles
use the DVE clamp trick (r = min(max(iota, c-1), c+1); window = r ==
iota, exact for integers) and 1/3 use ACT Square((y - c)^2) + one DVE
is_le.  Col-block pairs share a 2-bank psum tile; the count tile is
converted to 0/1 uint8 once per pair by ACT Sign (3/4) or DVE min
(1/4).  The uint8 band DMAs out (2 MiB per core; values are exactly
0/1 so u8 is lossless) and the host converts u8 -> f32.
"""

import numpy as np

import concourse.bass as bass
import concourse.mybir as mybir
import concourse.tile as tile
from concourse.bass_utils import run_bass_kernel_spmd
from concourse.vector_clock import ScopedClock

# Walrus in this container rejects instructions with >couple sem waits.
_MAX_WAITS = 1


def _split_drain_and_barrier(self, tick_clock, wait_clock):
    drain_inst = self.nc.sync.drain()
    wait_clock.add_sem_waits(
        drain_inst.ins, ScopedClock({None: tick_clock.global_clock})
    )
    si = drain_inst.ins.sync_info
    waits = list(si.on_wait) if si is not None and si.on_wait else []
    if len(waits) > _MAX_WAITS:
        si.on_wait = waits[:_MAX_WAITS]
        # spread the excess waits across engines so they drain in parallel
        engs = [self.nc.vector, self.nc.scalar, self.nc.tensor, self.nc.gpsimd,
                self.nc.sync]
        for n, i in enumerate(range(_MAX_WAITS, len(waits), _MAX_WAITS)):
            d = engs[n % len(engs)].drain().ins
            dsi = d.sync_info
            if dsi is None:
                d.sync_info = mybir.SyncInfo(
                    on_wait=waits[i : i + _MAX_WAITS], on_update=[]
                )
            else:
                dsi.on_wait = waits[i : i + _MAX_WAITS]

    self.nc.all_engine_barrier()
    assert self.sems is not None
    popped = self.nc._tile_sem_poison_stack.pop()
    assert popped is self._sem_poison
    self.nc.clear_and_free_semaphores(list(self.sems.allocated().values()))
    self.nc.all_engine_barrier()


tile.TileContext._drain_and_barrier = _split_drain_and_barrier


def _split_excess_waits(nc: bass.Bass, max_waits: int = _MAX_WAITS) -> None:
    for f in nc.m.functions:
        for bb in f.blocks:
            out = []
            for inst in bb.instructions:
                si = inst.sync_info
                waits = list(si.on_wait) if si is not None and si.on_wait else []
                if len(waits) > max_waits:
                    for i in range(max_waits, len(waits), max_waits):
                        d = mybir.InstEventSemaphore(
                            name=f"{inst.name}_swait{i}", ins=[], outs=[]
                        )
                        d.engine = inst.engine
                        d.sync_info = mybir.SyncInfo(
                            on_wait=waits[i : i + max_waits], on_update=[]
                        )
                        out.append(d)
                    si.on_wait = waits[:max_waits]
                out.append(inst)
            bb.instructions = out


W = 4096
H = 4096
M = 8                      # cores
BAND = W // M              # 512 rows per band
NTOT = BAND * H            # u8 bytes per band
NRC = 4                    # 128-row chunks per band
BW = 512                   # col-block width
NCB = H // BW              # 8 col-blocks
KT = 128                   # points per K-tile
PAD_C = 30000.0            # padded center: window lands nowhere
A = mybir.AluOpType

_build_cache: dict[tuple, bass.Bass] = {}


def _build(ktiles: tuple) -> bass.Bass:
    if ktiles in _build_cache:
        return _build_cache[ktiles]

    nkt = int(sum(ktiles))
    nc = bass.Bass("TRN2", target_bir_lowering=False, debug=False, num_devices=M)

    tbl_d = nc.dram_tensor("tbl", [128, 3 * nkt], mybir.dt.float32, kind="ExternalInput")
    ap_d = nc.dram_tensor("apm", [128, nkt * KT], mybir.dt.float16, kind="ExternalInput")
    out_d = nc.dram_tensor("out", [NTOT], mybir.dt.uint8, kind="ExternalOutput")
    zview = out_d.ap().rearrange("(c p f) -> c p f", p=128, f=H)

    with tile.TileContext(nc) as tc:
        with (
            tc.tile_pool(name="const", bufs=1) as cpool,
            tc.tile_pool(name="d", bufs=4) as dpool,
            tc.tile_pool(name="a", bufs=6) as apool,
            tc.tile_pool(name="band", bufs=2) as bpool,
            tc.tile_pool(name="psum", bufs=4, space="PSUM") as ppool,
        ):
            tbl = cpool.tile([128, 3 * nkt], mybir.dt.float32, tag="tbl")
            nc.sync.dma_start(tbl[:], tbl_d.ap())
            ap_all = cpool.tile([128, nkt * KT], mybir.dt.float16, tag="apm")
            nc.sync.dma_start(ap_all[:], ap_d.ap())
            lo_y = tbl[:, 0 * nkt : 1 * nkt]
            hi_y = tbl[:, 1 * nkt : 2 * nkt]
            cneg_y = tbl[:, 2 * nkt : 3 * nkt]

            iota_y = cpool.tile([128, BW], mybir.dt.float16, tag="ioy")
            iota_yf = cpool.tile([128, BW], mybir.dt.float32, tag="ioyf")
            nc.gpsimd.iota(iota_y[:], [[1, BW]], base=0, channel_multiplier=0, allow_small_or_imprecise_dtypes=True)
            nc.gpsimd.iota(iota_yf[:], [[1, BW]], base=0, channel_multiplier=0, allow_small_or_imprecise_dtypes=True)

            t = 0
            evict = 0
            for rc in range(NRC):
                band = bpool.tile([128, H], mybir.dt.uint8, tag="band")
                for cbp in range(NCB // 2):
                    # two col-blocks share one 2-bank psum tile so the
                    # count->u8 eviction runs once at double width
                    psum = ppool.tile([128, 2 * BW], mybir.dt.float32, tag="psum")
                    for half in range(2):
                        cb = 2 * cbp + half
                        pview = psum[:, half * BW : (half + 1) * BW]
                        nt = ktiles[rc * NCB + cb]
                        for j in range(nt):
                            ay = apool.tile([128, BW], mybir.dt.float16, tag="ay")
                            if t % 3 == 2:
                                # ACT path: d2 = (y-c)^2 on Scalar + DVE isle
                                d2 = dpool.tile([128, BW], mybir.dt.bfloat16, tag="d2")
                                nc.scalar.activation(
                                    d2[:], iota_yf[:],
                                    mybir.ActivationFunctionType.Square,
                                    bias=cneg_y[:, t : t + 1],
                                )
                                nc.vector.tensor_scalar(
                                    ay[:], d2[:], 2.25, None, A.is_le
                                )
                            else:
                                # DVE path: clamp trick; window membership is
                                # min(max(iota, lo), hi) == iota (all-integer)
                                ry = dpool.tile([128, BW], mybir.dt.float16, tag="ry")
                                nc.vector.tensor_scalar(
                                    ry[:], iota_y[:],
                                    lo_y[:, t : t + 1], hi_y[:, t : t + 1],
                                    A.max, A.min,
                                )
                                nc.vector.tensor_tensor(
                                    out=ay[:], in0=ry[:], in1=iota_y[:],
                                    op=A.is_equal,
                                )
                            nc.tensor.matmul(
                                pview, ap_all[:, t * KT : (t + 1) * KT], ay[:],
                                start=(j == 0), stop=(j == nt - 1),
                            )
                            t += 1
                    dst = band[:, cbp * 2 * BW : (cbp + 1) * 2 * BW]
                    if evict % 4 == 3:
                        nc.vector.tensor_scalar_min(dst, psum[:], 1.0)
                    else:
                        nc.scalar.activation(
                            dst, psum[:], mybir.ActivationFunctionType.Sign
                        )
                    evict += 1
                nc.sync.dma_start(zview[rc], band[:])
            assert t == nkt

    _split_excess_waits(nc)
    nc.finalize()
    _build_cache[ktiles] = nc
    return nc


def _prep(boxes: np.ndarray):
    """Points -> per-(core, chunk, col-block) K-tiled center tables."""
    cx = (boxes[:, 0] * W).astype(np.int64)
    cy = (boxes[:, 1] * H).astype(np.int64)

    # replicate each point to every 128-row chunk its 3-row window hits
    ch0 = (cx - 1) // 128
    ch1 = (cx + 1) // 128
    xs = [cx, cx[ch1 != ch0]]
    ys = [cy, cy[ch1 != ch0]]
    chs = [ch0, ch1[ch1 != ch0]]
    gx = np.concatenate(xs)
    gy = np.concatenate(ys)
    gch = np.concatenate(chs)
    keep = (gch >= 0) & (gch < 32)
    gx, gy, gch = gx[keep], gy[keep], gch[keep]

    # ... and to every 512-col block its 3-col window hits
    cb0 = (gy - 1) // BW
    cb1 = (gy + 1) // BW
    sx = np.concatenate([gx, gx[cb1 != cb0]])
    sy = np.concatenate([gy, gy[cb1 != cb0]])
    sch = np.concatenate([gch, gch[cb1 != cb0]])
    scb = np.concatenate([cb0, cb1[cb1 != cb0]])
    keep = (scb >= 0) & (scb < NCB)
    sx, sy, sch, scb = sx[keep], sy[keep], sch[keep], scb[keep]

    core = sch // NRC
    rc = sch % NRC
    pxl = sx - sch * 128          # in [-1, 128]
    cyl = sy - scb * BW           # in [-1, BW]

    blk = (rc * NCB + scb).astype(np.int64)
    buckets = {}
    counts = np.zeros((M, NRC * NCB), dtype=np.int64)
    for m in range(M):
        on = core == m
        bm, pm, cm = blk[on], pxl[on], cyl[on]
        order = np.argsort(bm, kind="stable")
        bm, pm, cm = bm[order], pm[order], cm[order]
        edges = np.searchsorted(bm, np.arange(NRC * NCB + 1))
        for b in range(NRC * NCB):
            sl = slice(edges[b], edges[b + 1])
            buckets[(m, b)] = (pm[sl], cm[sl])
            counts[m, b] = edges[b + 1] - edges[b]

    ktiles = tuple(
        max(1, int(-(-counts[:, b].max() // KT))) for b in range(NRC * NCB)
    )
    nkt = int(sum(ktiles))

    starts = np.zeros(NRC * NCB, dtype=np.int64)
    acc = 0
    for b in range(NRC * NCB):
        starts[b] = acc
        acc += ktiles[b]

    prng = np.arange(KT)
    in_maps = []
    for m in range(M):
        lo_y = np.full((128, nkt), PAD_C, dtype=np.float32)
        hi_y = np.full((128, nkt), PAD_C + 2, dtype=np.float32)
        cneg = np.full((128, nkt), -3.0 * PAD_C, dtype=np.float32)
        apm = np.zeros((nkt, 128, KT), dtype=np.float16)  # [tile, k, p]
        for b in range(NRC * NCB):
            pm, cm = buckets[(m, b)]
            n = pm.size
            t0 = starts[b]
            for j in range(int(ktiles[b])):
                lo, hi = j * KT, min((j + 1) * KT, n)
                if lo >= n:
                    break
                k = hi - lo
                lo_y[:k, t0 + j] = cm[lo:hi] - 1
                hi_y[:k, t0 + j] = cm[lo:hi] + 1
                cneg[:k, t0 + j] = -cm[lo:hi]
                apm[t0 + j, :k, :] = (
                    np.abs(prng[None, :] - pm[lo:hi, None]) <= 1
                ).astype(np.float16)
        tbl = np.concatenate([lo_y, hi_y, cneg], axis=1).astype(np.float32)
        # device layout [128 (k), nkt*KT (t, p)]
        ap_dev = np.ascontiguousarray(apm.transpose(1, 0, 2)).reshape(128, nkt * KT)
        in_maps.append({"tbl": tbl, "apm": ap_dev})
    return ktiles, in_maps


def _run(boxes: np.ndarray, trace: bool = False, **kwargs):
    boxes = np.asarray(boxes, dtype=np.float32)
    ktiles, in_maps = _prep(boxes)
    nc = _build(ktiles)
    res = run_bass_kernel_spmd(nc, in_maps, list(range(M)), trace=trace, **kwargs)
    bands = [
        np.asarray(res.results[m]["out"]).view(np.uint8).reshape(BAND, H)
        for m in range(M)
    ]
    img = np.concatenate(bands, axis=0)
    return img.reshape(1, 1, W, H).astype(np.float32), res


def kernel(boxes: np.ndarray) -> np.ndarray:
    out, _ = _run(boxes)
    return out


# revision 4
# speedup vs baseline: 1.0120x; 1.0120x over previous
"""HardHeatMap Trainium2 kernel, v2 (3x3-blob matmul scatter).

Identity: scatter 1.0 at floor(boxes*4096) then 3x3/stride-1/pad-1 max
pool == every point paints a 3x3 blob (clipped at edges); a pixel is 1
iff some blob covers it.

Device algorithm per core (band of 512 rows = 4 chunks of 128 rows x
8 col-blocks of 512): each point is ONE matmul K-slot.  For a K-tile of
up to 128 points:
    ap3[k, p] = (|p - px_k| <= 1)   row window, HOST-prepared fp16 lhsT
    ay3[k, y] = (|y - cy_k| <= 1)   col window, built on device (rhs)
    psum[p, y] += ap3^T @ ay3       3x3 blob counts (f32)
ay3 is built two ways, interleaved to balance engines: 2/3 of K-tiles
use the DVE clamp trick (r = min(max(iota, c-1), c+1); window = r ==
iota, exact for integers) and 1/3 use ACT Square((y - c)^2) + one DVE
is_le.  Col-block pairs share a 2-bank psum tile; the count tile is
converted to 0/1 uint8 once per pair by ACT Sign (3/4) or DVE min
(1/4).  The uint8 band DMAs out (2 MiB per core; values are exactly
0/1 so u8 is lossless) and the host converts u8 -> f32.
"""

import numpy as np

import concourse.bass as bass
import concourse.mybir as mybir
import concourse.tile as tile
from concourse.bass_utils import run_bass_kernel_spmd
from concourse.vector_clock import ScopedClock

# Walrus in this container rejects instructions with >couple sem waits.
_MAX_WAITS = 1


def _split_drain_and_barrier(self, tick_clock, wait_clock):
    drain_inst = self.nc.sync.drain()
    wait_clock.add_sem_waits(
        drain_inst.ins, ScopedClock({None: tick_clock.global_clock})
    )
    si = drain_inst.ins.sync_info
    waits = list(si.on_wait) if si is not None and si.on_wait else []
    if len(waits) > _MAX_WAITS:
        si.on_wait = waits[:_MAX_WAITS]
        # spread the excess waits across engines so they drain in parallel
        engs = [self.nc.vector, self.nc.scalar, self.nc.tensor, self.nc.gpsimd,
                self.nc.sync]
        for n, i in enumerate(range(_MAX_WAITS, len(waits), _MAX_WAITS)):
            d = engs[n % len(engs)].drain().ins
            dsi = d.sync_info
            if dsi is None:
                d.sync_info = mybir.SyncInfo(
                    on_wait=waits[i : i + _MAX_WAITS], on_update=[]
                )
            else:
                dsi.on_wait = waits[i : i + _MAX_WAITS]

    self.nc.all_engine_barrier()
    assert self.sems is not None
    popped = self.nc._tile_sem_poison_stack.pop()
    assert popped is self._sem_poison
    self.nc.clear_and_free_semaphores(list(self.sems.allocated().values()))
    self.nc.all_engine_barrier()


tile.TileContext._drain_and_barrier = _split_drain_and_barrier


def _split_excess_waits(nc: bass.Bass, max_waits: int = _MAX_WAITS) -> None:
    for f in nc.m.functions:
        for bb in f.blocks:
            out = []
            for inst in bb.instructions:
                si = inst.sync_info
                waits = list(si.on_wait) if si is not None and si.on_wait else []
                if len(waits) > max_waits:
                    for i in range(max_waits, len(waits), max_waits):
                        d = mybir.InstEventSemaphore(
                            name=f"{inst.name}_swait{i}", ins=[], outs=[]
                        )
                        d.engine = inst.engine
                        d.sync_info = mybir.SyncInfo(
                            on_wait=waits[i : i + max_waits], on_update=[]
                        )
                        out.append(d)
                    si.on_wait = waits[:max_waits]
                out.append(inst)
            bb.instructions = out


W = 4096
H = 4096
M = 8                      # cores
BAND = W // M              # 512 rows per band
NTOT = BAND * H            # u8 bytes per band
NRC = 4                    # 128-row chunks per band
BW = 512                   # col-block width
NCB = H // BW              # 8 col-blocks
KT = 128                   # points per K-tile
PAD_C = 30000.0            # padded center: window lands nowhere
A = mybir.AluOpType

_build_cache: dict[tuple, bass.Bass] = {}


def _build(ktiles: tuple) -> bass.Bass:
    if ktiles in _build_cache:
        return _build_cache[ktiles]

    nkt = int(sum(ktiles))
    nc = bass.Bass("TRN2", target_bir_lowering=False, debug=False, num_devices=M)

    tbl_d = nc.dram_tensor("tbl", [128, 3 * nkt], mybir.dt.float32, kind="ExternalInput")
    ap_d = nc.dram_tensor("apm", [128, nkt * KT], mybir.dt.float16, kind="ExternalInput")
    out_d = nc.dram_tensor("out", [NTOT], mybir.dt.uint8, kind="ExternalOutput")
    zview = out_d.ap().rearrange("(c p f) -> c p f", p=128, f=H)

    with tile.TileContext(nc) as tc:
        with (
            tc.tile_pool(name="const", bufs=1) as cpool,
            tc.tile_pool(name="d", bufs=4) as dpool,
            tc.tile_pool(name="a", bufs=6) as apool,
            tc.tile_pool(name="band", bufs=2) as bpool,
            tc.tile_pool(name="psum", bufs=4, space="PSUM") as ppool,
        ):
            tbl = cpool.tile([128, 3 * nkt], mybir.dt.float32, tag="tbl")
            nc.sync.dma_start(tbl[:], tbl_d.ap())
            ap_all = cpool.tile([128, nkt * KT], mybir.dt.float16, tag="apm")
            nc.sync.dma_start(ap_all[:], ap_d.ap())
            lo_y = tbl[:, 0 * nkt : 1 * nkt]
            hi_y = tbl[:, 1 * nkt : 2 * nkt]
            cneg_y = tbl[:, 2 * nkt : 3 * nkt]

            iota_y = cpool.tile([128, BW], mybir.dt.float16, tag="ioy")
            iota_yf = cpool.tile([128, BW], mybir.dt.float32, tag="ioyf")
            nc.gpsimd.iota(iota_y[:], [[1, BW]], base=0, channel_multiplier=0, allow_small_or_imprecise_dtypes=True)
            nc.gpsimd.iota(iota_yf[:], [[1, BW]], base=0, channel_multiplier=0, allow_small_or_imprecise_dtypes=True)

            t = 0
            evict = 0
            for rc in range(NRC):
                band = bpool.tile([128, H], mybir.dt.uint8, tag="band")
                for cbp in range(NCB // 2):
                    # two col-blocks share one 2-bank psum tile so the
                    # count->u8 eviction runs once at double width
                    psum = ppool.tile([128, 2 * BW], mybir.dt.float32, tag="psum")
                    for half in range(2):
                        cb = 2 * cbp + half
                        pview = psum[:, half * BW : (half + 1) * BW]
                        nt = ktiles[rc * NCB + cb]
                        for j in range(nt):
                            ay = apool.tile([128, BW], mybir.dt.float16, tag="ay")
                            if t % 3 == 2:
                                # ACT path: d2 = (y-c)^2 on Scalar + DVE isle
                                d2 = dpool.tile([128, BW], mybir.dt.bfloat16, tag="d2")
                                nc.scalar.activation(
                                    d2[:], iota_yf[:],
                                    mybir.ActivationFunctionType.Square,
                                    bias=cneg_y[:, t : t + 1],
                                )
                                nc.vector.tensor_scalar(
                                    ay[:], d2[:], 2.25, None, A.is_le
                                )
                            else:
                                # DVE path: clamp trick; window membership is
                                # min(max(iota, lo), hi) == iota (all-integer)
                                ry = dpool.tile([128, BW], mybir.dt.float16, tag="ry")
                                nc.vector.tensor_scalar(
                                    ry[:], iota_y[:],
                                    lo_y[:, t : t + 1], hi_y[:, t : t + 1],
                                    A.max, A.min,
                                )
                                nc.vector.tensor_tensor(
                                    out=ay[:], in0=ry[:], in1=iota_y[:],
                                    op=A.is_equal,
                                )
                            nc.tensor.matmul(
                                pview, ap_all[:, t * KT : (t + 1) * KT], ay[:],
                                start=(j == 0), stop=(j == nt - 1),
                            )
                            t += 1
                    dst = band[:, cbp * 2 * BW : (cbp + 1) * 2 * BW]
                    if evict % 8 == 7:
                        nc.vector.tensor_scalar_min(dst, psum[:], 1.0)
                    else:
                        nc.scalar.activation(
                            dst, psum[:], mybir.ActivationFunctionType.Sign
                        )
                    evict += 1
                nc.sync.dma_start(zview[rc], band[:])
            assert t == nkt

    _split_excess_waits(nc)
    nc.finalize()
    _build_cache[ktiles] = nc
    return nc


def _prep(boxes: np.ndarray):
    """Points -> per-(core, chunk, col-block) K-tiled center tables."""
    cx = (boxes[:, 0] * W).astype(np.int64)
    cy = (boxes[:, 1] * H).astype(np.int64)

    # replicate each point to every 128-row chunk its 3-row window hits
    ch0 = (cx - 1) // 128
    ch1 = (cx + 1) // 128
    xs = [cx, cx[ch1 != ch0]]
    ys = [cy, cy[ch1 != ch0]]
    chs = [ch0, ch1[ch1 != ch0]]
    gx = np.concatenate(xs)
    gy = np.concatenate(ys)
    gch = np.concatenate(chs)
    keep = (gch >= 0) & (gch < 32)
    gx, gy, gch = gx[keep], gy[keep], gch[keep]

    # ... and to every 512-col block its 3-col window hits
    cb0 = (gy - 1) // BW
    cb1 = (gy + 1) // BW
    sx = np.concatenate([gx, gx[cb1 != cb0]])
    sy = np.concatenate([gy, gy[cb1 != cb0]])
    sch = np.concatenate([gch, gch[cb1 != cb0]])
    scb = np.concatenate([cb0, cb1[cb1 != cb0]])
    keep = (scb >= 0) & (scb < NCB)
    sx, sy, sch, scb = sx[keep], sy[keep], sch[keep], scb[keep]

    core = sch // NRC
    rc = sch % NRC
    pxl = sx - sch * 128          # in [-1, 128]
    cyl = sy - scb * BW           # in [-1, BW]

    blk = (rc * NCB + scb).astype(np.int64)
    buckets = {}
    counts = np.zeros((M, NRC * NCB), dtype=np.int64)
    for m in range(M):
        on = core == m
        bm, pm, cm = blk[on], pxl[on], cyl[on]
        order = np.argsort(bm, kind="stable")
        bm, pm, cm = bm[order], pm[order], cm[order]
        edges = np.searchsorted(bm, np.arange(NRC * NCB + 1))
        for b in range(NRC * NCB):
            sl = slice(edges[b], edges[b + 1])
            buckets[(m, b)] = (pm[sl], cm[sl])
            counts[m, b] = edges[b + 1] - edges[b]

    ktiles = tuple(
        max(1, int(-(-counts[:, b].max() // KT))) for b in range(NRC * NCB)
    )
    nkt = int(sum(ktiles))

    starts = np.zeros(NRC * NCB, dtype=np.int64)
    acc = 0
    for b in range(NRC * NCB):
        starts[b] = acc
        acc += ktiles[b]

    prng = np.arange(KT)
    in_maps = []
    for m in range(M):
        lo_y = np.full((128, nkt), PAD_C, dtype=np.float32)
        hi_y = np.full((128, nkt), PAD_C + 2, dtype=np.float32)
        cneg = np.full((128, nkt), -3.0 * PAD_C, dtype=np.float32)
        apm = np.zeros((nkt, 128, KT), dtype=np.float16)  # [tile, k, p]
        for b in range(NRC * NCB):
            pm, cm = buckets[(m, b)]
            n = pm.size
            t0 = starts[b]
            for j in range(int(ktiles[b])):
                lo, hi = j * KT, min((j + 1) * KT, n)
                if lo >= n:
                    break
                k = hi - lo
                lo_y[:k, t0 + j] = cm[lo:hi] - 1
                hi_y[:k, t0 + j] = cm[lo:hi] + 1
                cneg[:k, t0 + j] = -cm[lo:hi]
                apm[t0 + j, :k, :] = (
                    np.abs(prng[None, :] - pm[lo:hi, None]) <= 1
                ).astype(np.float16)
        tbl = np.concatenate([lo_y, hi_y, cneg], axis=1).astype(np.float32)
        # device layout [128 (k), nkt*KT (t, p)]
        ap_dev = np.ascontiguousarray(apm.transpose(1, 0, 2)).reshape(128, nkt * KT)
        in_maps.append({"tbl": tbl, "apm": ap_dev})
    return ktiles, in_maps


def _run(boxes: np.ndarray, trace: bool = False, **kwargs):
    boxes = np.asarray(boxes, dtype=np.float32)
    ktiles, in_maps = _prep(boxes)
    nc = _build(ktiles)
    res = run_bass_kernel_spmd(nc, in_maps, list(range(M)), trace=trace, **kwargs)
    bands = [
        np.asarray(res.results[m]["out"]).view(np.uint8).reshape(BAND, H)
        for m in range(M)
    ]
    img = np.concatenate(bands, axis=0)
    return img.reshape(1, 1, W, H).astype(np.float32), res


def kernel(boxes: np.ndarray) -> np.ndarray:
    out, _ = _run(boxes)
    return out


# revision 5
# speedup vs baseline: 1.0465x; 1.0340x over previous
"""HardHeatMap Trainium2 kernel, v2 (3x3-blob matmul scatter).

Identity: scatter 1.0 at floor(boxes*4096) then 3x3/stride-1/pad-1 max
pool == every point paints a 3x3 blob (clipped at edges); a pixel is 1
iff some blob covers it.

Device algorithm per core (band of 512 rows = 4 chunks of 128 rows x
8 col-blocks of 512): each point is ONE matmul K-slot.  For a K-tile of
up to 128 points:
    ap3[k, p] = (|p - px_k| <= 1)   row window, HOST-prepared fp16 lhsT
    ay3[k, y] = (|y - cy_k| <= 1)   col window, built on device (rhs)
    psum[p, y] += ap3^T @ ay3       3x3 blob counts (f32)
ay3 is built two ways, interleaved to balance engines: 2/3 of K-tiles
use the DVE clamp trick (r = min(max(iota, c-1), c+1); window = r ==
iota, exact for integers) and 1/3 use ACT Square((y - c)^2) + one DVE
is_le.  Col-block pairs share a 2-bank psum tile; the count tile is
converted to 0/1 uint8 once per pair by ACT Sign (3/4) or DVE min
(1/4).  The uint8 band DMAs out (2 MiB per core; values are exactly
0/1 so u8 is lossless) and the host converts u8 -> f32.
"""

import numpy as np

import concourse.bass as bass
import concourse.mybir as mybir
import concourse.tile as tile
from concourse.bass_utils import run_bass_kernel_spmd
from concourse.vector_clock import ScopedClock

# Walrus in this container rejects instructions with >couple sem waits.
_MAX_WAITS = 1


def _split_drain_and_barrier(self, tick_clock, wait_clock):
    drain_inst = self.nc.sync.drain()
    wait_clock.add_sem_waits(
        drain_inst.ins, ScopedClock({None: tick_clock.global_clock})
    )
    si = drain_inst.ins.sync_info
    waits = list(si.on_wait) if si is not None and si.on_wait else []
    if len(waits) > _MAX_WAITS:
        si.on_wait = waits[:_MAX_WAITS]
        # spread the excess waits across engines so they drain in parallel
        engs = [self.nc.vector, self.nc.scalar, self.nc.tensor, self.nc.gpsimd,
                self.nc.sync]
        for n, i in enumerate(range(_MAX_WAITS, len(waits), _MAX_WAITS)):
            d = engs[n % len(engs)].drain().ins
            dsi = d.sync_info
            if dsi is None:
                d.sync_info = mybir.SyncInfo(
                    on_wait=waits[i : i + _MAX_WAITS], on_update=[]
                )
            else:
                dsi.on_wait = waits[i : i + _MAX_WAITS]

    self.nc.all_engine_barrier()
    assert self.sems is not None
    popped = self.nc._tile_sem_poison_stack.pop()
    assert popped is self._sem_poison
    self.nc.clear_and_free_semaphores(list(self.sems.allocated().values()))
    self.nc.all_engine_barrier()


tile.TileContext._drain_and_barrier = _split_drain_and_barrier


def _split_excess_waits(nc: bass.Bass, max_waits: int = _MAX_WAITS) -> None:
    for f in nc.m.functions:
        for bb in f.blocks:
            out = []
            for inst in bb.instructions:
                si = inst.sync_info
                waits = list(si.on_wait) if si is not None and si.on_wait else []
                if len(waits) > max_waits:
                    for i in range(max_waits, len(waits), max_waits):
                        d = mybir.InstEventSemaphore(
                            name=f"{inst.name}_swait{i}", ins=[], outs=[]
                        )
                        d.engine = inst.engine
                        d.sync_info = mybir.SyncInfo(
                            on_wait=waits[i : i + max_waits], on_update=[]
                        )
                        out.append(d)
                    si.on_wait = waits[:max_waits]
                out.append(inst)
            bb.instructions = out


W = 4096
H = 4096
M = 8                      # cores
BAND = W // M              # 512 rows per band
NTOT = BAND * H            # u8 bytes per band
NRC = 4                    # 128-row chunks per band
BW = 512                   # col-block width
NCB = H // BW              # 8 col-blocks
KT = 128                   # points per K-tile
PAD_C = 30000.0            # padded center: window lands nowhere
A = mybir.AluOpType

_build_cache: dict[tuple, bass.Bass] = {}


def _build(ktiles: tuple) -> bass.Bass:
    if ktiles in _build_cache:
        return _build_cache[ktiles]

    nkt = int(sum(ktiles))
    nc = bass.Bass("TRN2", target_bir_lowering=False, debug=False, num_devices=M)

    tbl_d = nc.dram_tensor("tbl", [128, 3 * nkt], mybir.dt.float32, kind="ExternalInput")
    ap_d = nc.dram_tensor("apm", [128, nkt * KT], mybir.dt.float16, kind="ExternalInput")
    out_d = nc.dram_tensor("out", [NTOT], mybir.dt.uint8, kind="ExternalOutput")
    zview = out_d.ap().rearrange("(c p f) -> c p f", p=128, f=H)

    with tile.TileContext(nc) as tc:
        with (
            tc.tile_pool(name="const", bufs=1) as cpool,
            tc.tile_pool(name="d", bufs=4) as dpool,
            tc.tile_pool(name="a", bufs=6) as apool,
            tc.tile_pool(name="band", bufs=2) as bpool,
            tc.tile_pool(name="psum", bufs=4, space="PSUM") as ppool,
        ):
            tbl = cpool.tile([128, 3 * nkt], mybir.dt.float32, tag="tbl")
            nc.sync.dma_start(tbl[:], tbl_d.ap())
            ap_all = cpool.tile([128, nkt * KT], mybir.dt.float16, tag="apm")
            nc.sync.dma_start(ap_all[:], ap_d.ap())
            lo_y = tbl[:, 0 * nkt : 1 * nkt]
            hi_y = tbl[:, 1 * nkt : 2 * nkt]
            cneg_y = tbl[:, 2 * nkt : 3 * nkt]

            iota_y = cpool.tile([128, BW], mybir.dt.float16, tag="ioy")
            iota_yf = cpool.tile([128, BW], mybir.dt.float32, tag="ioyf")
            nc.gpsimd.iota(iota_y[:], [[1, BW]], base=0, channel_multiplier=0, allow_small_or_imprecise_dtypes=True)
            nc.gpsimd.iota(iota_yf[:], [[1, BW]], base=0, channel_multiplier=0, allow_small_or_imprecise_dtypes=True)

            t = 0
            evict = 0
            for rc in range(NRC):
                band = bpool.tile([128, H], mybir.dt.uint8, tag="band")
                for cbp in range(NCB // 2):
                    # two col-blocks share one 2-bank psum tile so the
                    # count->u8 eviction runs once at double width
                    psum = ppool.tile([128, 2 * BW], mybir.dt.float32, tag="psum")
                    for half in range(2):
                        cb = 2 * cbp + half
                        pview = psum[:, half * BW : (half + 1) * BW]
                        nt = ktiles[rc * NCB + cb]
                        for j in range(nt):
                            ay = apool.tile([128, BW], mybir.dt.float16, tag="ay")
                            if t % 3 == 2:
                                # ACT path: d2 = (y-c)^2 on Scalar + DVE isle
                                d2 = dpool.tile([128, BW], mybir.dt.bfloat16, tag="d2")
                                nc.scalar.activation(
                                    d2[:], iota_yf[:],
                                    mybir.ActivationFunctionType.Square,
                                    bias=cneg_y[:, t : t + 1],
                                )
                                nc.vector.tensor_scalar(
                                    ay[:], d2[:], 2.25, None, A.is_le
                                )
                            else:
                                # DVE path: clamp trick; window membership is
                                # min(max(iota, lo), hi) == iota (all-integer)
                                ry = dpool.tile([128, BW], mybir.dt.float16, tag="ry")
                                nc.vector.tensor_scalar(
                                    ry[:], iota_y[:],
                                    lo_y[:, t : t + 1], hi_y[:, t : t + 1],
                                    A.max, A.min,
                                )
                                nc.vector.tensor_tensor(
                                    out=ay[:], in0=ry[:], in1=iota_y[:],
                                    op=A.is_equal,
                                )
                            nc.tensor.matmul(
                                pview, ap_all[:, t * KT : (t + 1) * KT], ay[:],
                                start=(j == 0), stop=(j == nt - 1),
                            )
                            t += 1
                    dst = band[:, cbp * 2 * BW : (cbp + 1) * 2 * BW]
                    if evict % 8 == 7:
                        nc.vector.tensor_scalar_min(dst, psum[:], 1.0)
                    else:
                        nc.scalar.activation(
                            dst, psum[:], mybir.ActivationFunctionType.Sign
                        )
                    evict += 1
                nc.sync.dma_start(zview[rc][:, 0 : H // 2], band[:, 0 : H // 2])
                nc.sync.dma_start(zview[rc][:, H // 2 : H], band[:, H // 2 : H])
            assert t == nkt

    _split_excess_waits(nc)
    nc.finalize()
    _build_cache[ktiles] = nc
    return nc


def _prep(boxes: np.ndarray):
    """Points -> per-(core, chunk, col-block) K-tiled center tables."""
    cx = (boxes[:, 0] * W).astype(np.int64)
    cy = (boxes[:, 1] * H).astype(np.int64)

    # replicate each point to every 128-row chunk its 3-row window hits
    ch0 = (cx - 1) // 128
    ch1 = (cx + 1) // 128
    xs = [cx, cx[ch1 != ch0]]
    ys = [cy, cy[ch1 != ch0]]
    chs = [ch0, ch1[ch1 != ch0]]
    gx = np.concatenate(xs)
    gy = np.concatenate(ys)
    gch = np.concatenate(chs)
    keep = (gch >= 0) & (gch < 32)
    gx, gy, gch = gx[keep], gy[keep], gch[keep]

    # ... and to every 512-col block its 3-col window hits
    cb0 = (gy - 1) // BW
    cb1 = (gy + 1) // BW
    sx = np.concatenate([gx, gx[cb1 != cb0]])
    sy = np.concatenate([gy, gy[cb1 != cb0]])
    sch = np.concatenate([gch, gch[cb1 != cb0]])
    scb = np.concatenate([cb0, cb1[cb1 != cb0]])
    keep = (scb >= 0) & (scb < NCB)
    sx, sy, sch, scb = sx[keep], sy[keep], sch[keep], scb[keep]

    core = sch // NRC
    rc = sch % NRC
    pxl = sx - sch * 128          # in [-1, 128]
    cyl = sy - scb * BW           # in [-1, BW]

    blk = (rc * NCB + scb).astype(np.int64)
    buckets = {}
    counts = np.zeros((M, NRC * NCB), dtype=np.int64)
    for m in range(M):
        on = core == m
        bm, pm, cm = blk[on], pxl[on], cyl[on]
        order = np.argsort(bm, kind="stable")
        bm, pm, cm = bm[order], pm[order], cm[order]
        edges = np.searchsorted(bm, np.arange(NRC * NCB + 1))
        for b in range(NRC * NCB):
            sl = slice(edges[b], edges[b + 1])
            buckets[(m, b)] = (pm[sl], cm[sl])
            counts[m, b] = edges[b + 1] - edges[b]

    ktiles = tuple(
        max(1, int(-(-counts[:, b].max() // KT))) for b in range(NRC * NCB)
    )
    nkt = int(sum(ktiles))

    starts = np.zeros(NRC * NCB, dtype=np.int64)
    acc = 0
    for b in range(NRC * NCB):
        starts[b] = acc
        acc += ktiles[b]

    prng = np.arange(KT)
    in_maps = []
    for m in range(M):
        lo_y = np.full((128, nkt), PAD_C, dtype=np.float32)
        hi_y = np.full((128, nkt), PAD_C + 2, dtype=np.float32)
        cneg = np.full((128, nkt), -3.0 * PAD_C, dtype=np.float32)
        apm = np.zeros((nkt, 128, KT), dtype=np.float16)  # [tile, k, p]
        for b in range(NRC * NCB):
            pm, cm = buckets[(m, b)]
            n = pm.size
            t0 = starts[b]
            for j in range(int(ktiles[b])):
                lo, hi = j * KT, min((j + 1) * KT, n)
                if lo >= n:
                    break
                k = hi - lo
                lo_y[:k, t0 + j] = cm[lo:hi] - 1
                hi_y[:k, t0 + j] = cm[lo:hi] + 1
                cneg[:k, t0 + j] = -cm[lo:hi]
                apm[t0 + j, :k, :] = (
                    np.abs(prng[None, :] - pm[lo:hi, None]) <= 1
                ).astype(np.float16)
        tbl = np.concatenate([lo_y, hi_y, cneg], axis=1).astype(np.float32)
        # device layout [128 (k), nkt*KT (t, p)]
        ap_dev = np.ascontiguousarray(apm.transpose(1, 0, 2)).reshape(128, nkt * KT)
        in_maps.append({"tbl": tbl, "apm": ap_dev})
    return ktiles, in_maps


def _run(boxes: np.ndarray, trace: bool = False, **kwargs):
    boxes = np.asarray(boxes, dtype=np.float32)
    ktiles, in_maps = _prep(boxes)
    nc = _build(ktiles)
    res = run_bass_kernel_spmd(nc, in_maps, list(range(M)), trace=trace, **kwargs)
    bands = [
        np.asarray(res.results[m]["out"]).view(np.uint8).reshape(BAND, H)
        for m in range(M)
    ]
    img = np.concatenate(bands, axis=0)
    return img.reshape(1, 1, W, H).astype(np.float32), res


def kernel(boxes: np.ndarray) -> np.ndarray:
    out, _ = _run(boxes)
    return out
